# revision 2
# baseline (speedup 1.0000x reference)
"""MoE kernel v4: every core holds an F/8 slice of ALL 8 experts.

Each of the 8 cores owns columns [c*512, (c+1)*512) of the hidden dim F
for every expert, and processes ALL 16384 (token, expert) pairs on that
slice: h = gelu(x @ w1_slice + b1_slice); y_partial = h @ w2_slice.
The 8 partial outputs (each full D) are summed on host, then combined
with the router weights and scattered back — so per-core PE work is
exactly total/8 (perfect balance for any routing), with zero padding:
16384 pairs = 32 tiles x 512 columns exactly.

Token-expert pairs are sorted by expert; a tile that spans an expert
boundary is split into (at most 2) column segments, each matmul'd with
its own expert's weights. The segment layout is identical on every core
(same routing), so one SPMD program serves all cores; it is compiled
per routing pattern and cached.

DRAM layouts per core (KO = D/128 = 8, FS = F/8 = 512):
  x  [32, 128, KO, 512] bf16   x[t,p,ko,j] = xf[tok(t,j), ko*128+p]   (same all cores)
  w1 [8, 128, 4, KO, 128] bf16 w1[e,p,fo,ko,m] = w1_e[ko*128+p, c*FS+fo*128+m]
  w2 [8, 128, 4, 1024] bf16    w2[e,p,fi,d]   = w2_e[c*FS+fi*128+p, d]
  b1 [128, 8, 4] f32           b1[p,e,fo]     = b1_e[c*FS+fo*128+p]
  y  [32, 128, KO, 512] bf16   partial (gelu(x@w1s+b1s) @ w2s)^T

A few dummy matmuls on zeroed scratch SBUF run at the head of the PE
stream (no DMA deps) to warm the HAM clock gate and cover the initial
DMA window; DMAs are emitted smallest-needed-first so real matmuls
start as early as possible.
"""

import numpy as np
import ml_dtypes

N_CORES = 8
D = 1024
F = 4096
E = 8
KO = D // 128          # 8 k-chunks of the model dim
FS = F // N_CORES      # 512 local F columns per core
FO = FS // 128         # 4 local f-chunks
CT = 512               # tile width (token-expert pairs per tile)
N_TILES = 32           # 16384 / 512, exact
N_WARM = 10

BF16 = ml_dtypes.bfloat16

_NC_CACHE: dict[tuple, object] = {}
LAST_RESULTS = None


def _build(spec):
    """spec: tuple over tiles of tuple of (expert, c0, c1) segments."""
    import concourse.mybir as mybir
    from concourse import bacc
    from concourse.tile import TileContext

    fp32 = mybir.dt.float32
    bf16 = mybir.dt.bfloat16

    nc = bacc.Bacc(
        "TRN2", target_bir_lowering=False, debug=False, num_devices=N_CORES
    )
    x = nc.dram_tensor("x", [N_TILES, 128, KO, CT], bf16, kind="ExternalInput")
    w1 = nc.dram_tensor("w1", [E, 128, FO, KO, 128], bf16, kind="ExternalInput")
    w2 = nc.dram_tensor("w2", [E, 128, FO, D], bf16, kind="ExternalInput")
    b1 = nc.dram_tensor("b1", [128, E, FO], fp32, kind="ExternalInput")
    y = nc.dram_tensor("y", [N_TILES, 128, KO, CT], bf16, kind="ExternalOutput")

    # experts in order of first use, for weight DMA emission order
    e_order = []
    for segs in spec:
        for e, _, _ in segs:
            if e not in e_order:
                e_order.append(e)
    for e in range(E):
        if e not in e_order:
            e_order.append(e)

    with TileContext(nc) as tc:
        with (
            tc.tile_pool(name="wpool", bufs=1) as wpool,
            tc.tile_pool(name="xpool", bufs=4) as xpool,
            tc.tile_pool(name="hpool", bufs=2) as hpool,
            tc.tile_pool(name="ypool", bufs=4) as ypool,
            tc.tile_pool(name="ph", bufs=2, space="PSUM") as phpool,
            tc.tile_pool(name="py", bufs=3, space="PSUM") as pypool,
            tc.tile_pool(name="pw", bufs=1, space="PSUM") as pwpool,
        ):
            w1_sb = wpool.tile([128, E, FO, KO, 128], bf16)
            w2_sb = wpool.tile([128, E, FO, D], bf16)
            b1_sb = wpool.tile([128, E, FO], fp32)
            warm_x = wpool.tile([128, CT], bf16)

            # PE warm-up: matmuls with no DMA deps keep the PE busy (and
            # un-throttle the HAM clock gate) while the first DMAs land.
            nc.vector.memset(warm_x[:], 0.0)
            warm_ps = pwpool.tile([128, CT], fp32)
            for _ in range(N_WARM):
                nc.tensor.matmul(
                    warm_ps[:],
                    lhsT=warm_x[:, 0:128],
                    rhs=warm_x[:],
                    start=True,
                    stop=True,
                )

            # ---- DMA emission, smallest-needed-first ----
            e0 = e_order[0]
            x_first = xpool.tile([128, KO, CT], bf16, tag="x_sb")
            nc.sync.dma_start(x_first[:, 0:4, :], x[0][:, 0:4, :])
            nc.sync.dma_start(x_first[:, 4:8, :], x[0][:, 4:8, :])
            nc.sync.dma_start(w1_sb[:, e0, 0], w1[e0, :, 0])
            nc.sync.dma_start(b1_sb[:], b1[:])
            for fo in range(1, FO):
                nc.sync.dma_start(w1_sb[:, e0, fo], w1[e0, :, fo])
            nc.sync.dma_start(w2_sb[:, e0], w2[e0])
            x_pre = [x_first]
            for ti in range(1, 4):
                t = xpool.tile([128, KO, CT], bf16, tag="x_sb")
                nc.sync.dma_start(t[:], x[ti])
                x_pre.append(t)
            for e in e_order[1:]:
                nc.sync.dma_start(w1_sb[:, e], w1[e])
                nc.sync.dma_start(w2_sb[:, e], w2[e])

            # ---- main loop ----
            for ti in range(N_TILES):
                if ti < 4:
                    x_sb = x_pre[ti]
                else:
                    x_sb = xpool.tile([128, KO, CT], bf16, tag="x_sb")
                    nc.sync.dma_start(x_sb[:], x[ti])
                segs = spec[ti]
                h_sb = hpool.tile([128, FO, CT], bf16)
                for fo in range(FO):
                    ph = phpool.tile([128, CT], fp32)
                    for e, c0, c1 in segs:
                        for ko in range(KO):
                            nc.tensor.matmul(
                                ph[:, c0:c1],
                                lhsT=w1_sb[:, e, fo, ko],
                                rhs=x_sb[:, ko, c0:c1],
                                start=(ko == 0),
                                stop=(ko == KO - 1),
                            )
                    for e, c0, c1 in segs:
                        nc.scalar.activation(
                            h_sb[:, fo, c0:c1],
                            ph[:, c0:c1],
                            mybir.ActivationFunctionType.Gelu,
                            bias=b1_sb[:, e, fo : fo + 1],
                        )
                for dj in range(KO // 2):
                    y_sb = ypool.tile([128, 2, CT], bf16, tag="y_sb")
                    for dk in range(2):
                        do = dj * 2 + dk
                        py = pypool.tile([128, CT], fp32)
                        for e, c0, c1 in segs:
                            for fi in range(FO):
                                nc.tensor.matmul(
                                    py[:, c0:c1],
                                    lhsT=w2_sb[:, e, fi, do * 128 : (do + 1) * 128],
                                    rhs=h_sb[:, fi, c0:c1],
                                    start=(fi == 0),
                                    stop=(fi == FO - 1),
                                )
                        nc.vector.tensor_copy(y_sb[:, dk, :], py[:])
                    nc.sync.dma_start(y[ti][:, dj * 2 : dj * 2 + 2, :], y_sb[:])

    nc.compile()
    return nc


def kernel(x, gate_w, w1, b1, w2, b2):
    from concourse.bass_utils import run_bass_kernel_spmd

    global LAST_RESULTS

    x = np.asarray(x, dtype=np.float32)
    gate_w = np.asarray(gate_w, dtype=np.float32)
    w1 = np.asarray(w1, dtype=np.float32)
    b1 = np.asarray(b1, dtype=np.float32)
    w2 = np.asarray(w2, dtype=np.float32)
    b2 = np.asarray(b2, dtype=np.float32)

    B, S, Din = x.shape
    assert Din == D and gate_w.shape == (D, E)
    T = B * S
    xf = x.reshape(T, D)

    # ---- Host router ----
    logits = xf.astype(np.float64) @ gate_w.astype(np.float64)
    idx0 = np.argmax(logits, axis=1)
    rows = np.arange(T)
    v0 = logits[rows, idx0]
    l2 = logits.copy()
    l2[rows, idx0] = -np.inf
    idx1 = np.argmax(l2, axis=1)
    v1_ = l2[rows, idx1]
    e1 = np.exp(v1_ - v0)
    cw0 = 1.0 / (1.0 + e1)
    cw1 = e1 / (1.0 + e1)

    tok_stream = []
    cw_stream = []
    for e in range(E):
        sel0 = idx0 == e
        sel1 = idx1 == e
        ids = np.nonzero(sel0 | sel1)[0]
        w = np.where(sel0[ids], cw0[ids], cw1[ids])
        tok_stream.append(ids)
        cw_stream.append(w)
    counts = np.array([len(t) for t in tok_stream])
    assert counts.sum() == 2 * T == N_TILES * CT
    bounds = np.cumsum(counts)
    tok_stream = np.concatenate(tok_stream)
    cw_stream = np.concatenate(cw_stream).astype(np.float32)

    # Per-tile expert segments (same on every core)
    spec = []
    for ti in range(N_TILES):
        lo, hi = ti * CT, (ti + 1) * CT
        segs = []
        for e in range(E):
            s0 = 0 if e == 0 else int(bounds[e - 1])
            s1 = int(bounds[e])
            a, b = max(lo, s0), min(hi, s1)
            if a < b:
                segs.append((e, a - lo, b - lo))
        spec.append(tuple(segs))
    spec = tuple(spec)

    if spec not in _NC_CACHE:
        _NC_CACHE[spec] = _build(spec)
    nc = _NC_CACHE[spec]

    # ---- Pack inputs ----
    xd = np.ascontiguousarray(
        xf[tok_stream]
        .astype(BF16)
        .reshape(N_TILES, CT, KO, 128)
        .transpose(0, 3, 2, 1)
    )
    in_maps = []
    for c in range(N_CORES):
        sl = slice(c * FS, (c + 1) * FS)
        w1c = np.ascontiguousarray(
            np.stack(
                [
                    w1[e][:, sl].reshape(KO, 128, FO, 128).transpose(1, 2, 0, 3)
                    for e in range(E)
                ]
            ).astype(BF16)
        )  # [E, 128, FO, KO, 128]
        w2c = np.ascontiguousarray(
            np.stack(
                [
                    w2[e][sl, :].reshape(FO, 128, D).transpose(1, 0, 2)
                    for e in range(E)
                ]
            ).astype(BF16)
        )  # [E, 128, FO, D]
        b1c = np.ascontiguousarray(
            np.stack([b1[e][sl].reshape(FO, 128).T for e in range(E)], axis=1)
        )  # [128, E, FO]
        in_maps.append({"x": xd, "w1": w1c, "w2": w2c, "b1": b1c})

    res = run_bass_kernel_spmd(nc, in_maps, core_ids=list(range(N_CORES)))
    LAST_RESULTS = res

    # ---- Host combine: sum F-slice partials, weight, scatter ----
    ysum = np.zeros((N_TILES, 128, KO, CT), dtype=np.float32)
    for c in range(N_CORES):
        ysum += res.results[c]["y"].astype(np.float32)
    yt = ysum.transpose(0, 3, 2, 1).reshape(2 * T, D)
    exp_stream = np.repeat(np.arange(E), counts)
    contrib = cw_stream[:, None] * (yt + b2[exp_stream])
    out = np.zeros((T, D), dtype=np.float32)
    np.add.at(out, tok_stream, contrib)

    return out.reshape(B, S, D)


# revision 3
# speedup vs baseline: 1.3312x; 1.3312x over previous
"""MoE kernel v6: v3 pair structure + fp8 DoubleRow for low-weight jobs.

Structure (per pair of cores, F split in half as v3): each core holds the
F-half of BOTH its pair's experts. Jobs are split by router combine
weight: second-choice jobs with cw < THETA contribute little to the
output, so they run in fp8 e4m3 with DoubleRow perf mode (2 k-chunks per
matmul pass, ~1.5x faster); everything else stays bf16. Weights for the
fp8 path are pre-scaled by 128 on host (w values ~1/32 std would hit
e4m3 subnormals); the scale is undone by the activation's input scale
(mm1) and on host (mm2 output).

Phases per core: A (big expert, bf16) -> C (big cheap, fp8) -> D (small
cheap, fp8) -> B (small expert, bf16). B's bf16 weights load into the
SBUF region released by A's weights while the fp8 phases compute, so
both bf16 weight sets plus the fp8 set never need to be resident
together. A few warm-up matmuls on zeroed scratch SBUF run at the head
of the PE stream to cover the initial DMA window and warm the HAM clock
gate; DMAs are emitted smallest-needed-first.

Slot capacities (CA..CD) are maxima across pairs so the program is SPMD;
per-core data decides which experts a core serves.

DRAM layouts per core (FL = F/2 = 2048, FLO = 16, KO = 8):
  xa  [nA+nB, 128, KO, CT] bf16   A tiles then B tiles; xa[t,p,ko,j] = xf[tok, ko*128+p]
  xc  [nC+nD, 128, 4, 2, CT] f8e4 C tiles then D; xc[t,p,j,i,c] = xf[tok, (2j+i)*128+p]
  w1a/w1b [128, 4, KO, 512] bf16  w1_e[ko*128+p, h*FL+q*512+ff]
  w2a/w2b [2, 128, 8, D] bf16     w2_e[h*FL+(b*8+fi)*128+p, d]
  w1c [2, 128, FLO, 4, 2, 128] f8 128*w1_{e_s}[(2j+i)*128+p, h*FL+fo*128+m]
  w2c [2, 128, 8, 2, D] f8        128*w2_{e_s}[h*FL+(2fj+i)*128+p, d]
  b1a/b1b [128, FLO] f32; b1c [128, 2, FLO] f32 (true bias)
  y   [nA+nC+nD+nB, 128, KO, CT] bf16  partials; fp8-phase tiles carry 128x scale
"""

import numpy as np
import ml_dtypes

N_CORES = 8
D = 1024
F = 4096
E = 8
KO = D // 128
FL = F // 2
FLO = FL // 128      # 16
CT = 512
THETA = 0.38
SCALE = 128.0
N_WARM = 10

BF16 = ml_dtypes.bfloat16
F8 = ml_dtypes.float8_e4m3

_NC_CACHE: dict[tuple, object] = {}
LAST_RESULTS = None


def _cap_tiles(C):
    tiles = []
    off = 0
    while C - off >= CT:
        tiles.append((off, CT))
        off += CT
    if off < C:
        tiles.append((off, C - off))
    return tiles


def _build(caps):
    import concourse.mybir as mybir
    from concourse import bacc
    from concourse.tile import TileContext

    fp32 = mybir.dt.float32
    bf16 = mybir.dt.bfloat16
    f8e4 = mybir.dt.float8e4
    DR = mybir.MatmulPerfMode.DoubleRow

    CA, CB, CC, CD = caps
    tA, tB, tC, tD = (_cap_tiles(c) for c in caps)
    nA, nB, nC, nD = len(tA), len(tB), len(tC), len(tD)
    n_tot = nA + nB + nC + nD

    nc = bacc.Bacc(
        "TRN2", target_bir_lowering=False, debug=False, num_devices=N_CORES
    )
    xa = nc.dram_tensor("xa", [nA + nB, 128, KO, CT], bf16, kind="ExternalInput")
    xc = nc.dram_tensor("xc", [nC + nD, 128, 4, 2, CT], f8e4, kind="ExternalInput")
    w1a = nc.dram_tensor("w1a", [128, 4, KO, 512], bf16, kind="ExternalInput")
    w1b = nc.dram_tensor("w1b", [128, 4, KO, 512], bf16, kind="ExternalInput")
    w2a = nc.dram_tensor("w2a", [2, 128, 8, D], bf16, kind="ExternalInput")
    w2b = nc.dram_tensor("w2b", [2, 128, 8, D], bf16, kind="ExternalInput")
    w1c = nc.dram_tensor("w1c", [2, 128, FLO, 4, 2, 128], f8e4, kind="ExternalInput")
    w2c = nc.dram_tensor("w2c", [2, 128, 8, 2, D], f8e4, kind="ExternalInput")
    b1a = nc.dram_tensor("b1a", [128, FLO], fp32, kind="ExternalInput")
    b1b = nc.dram_tensor("b1b", [128, FLO], fp32, kind="ExternalInput")
    b1c = nc.dram_tensor("b1c", [128, 2, FLO], fp32, kind="ExternalInput")
    y = nc.dram_tensor("y", [n_tot, 128, KO, CT], bf16, kind="ExternalOutput")

    with TileContext(nc) as tc:
        with (
            tc.tile_pool(name="wg", bufs=1) as wg,
            tc.tile_pool(name="wcd", bufs=1) as wcd,
            tc.tile_pool(name="xpool", bufs=3) as xpool,
            tc.tile_pool(name="xcpool", bufs=2) as xcpool,
            tc.tile_pool(name="hpool", bufs=1) as hpool,
            tc.tile_pool(name="hcpool", bufs=1) as hcpool,
            tc.tile_pool(name="ypool", bufs=4) as ypool,
            tc.tile_pool(name="ph", bufs=3, space="PSUM") as phpool,
            tc.tile_pool(name="py", bufs=3, space="PSUM") as pypool,
            tc.tile_pool(name="pw", bufs=1, space="PSUM") as pwpool,
        ):
            b1a_sb = wg.tile([128, FLO], fp32)
            b1b_sb = wg.tile([128, FLO], fp32)
            b1c_sb = wg.tile([128, 2, FLO], fp32)
            warm_x = wg.tile([128, CT], bf16)
            w1c_sb = wcd.tile([128, 2, FLO, 4, 2, 128], f8e4)
            w2c_sb = wcd.tile([128, 2, 8, 2, D], f8e4)

            nc.vector.memset(warm_x[:], 0.0)
            warm_ps = pwpool.tile([128, CT], fp32)
            for _ in range(N_WARM):
                nc.tensor.matmul(
                    warm_ps[:],
                    lhsT=warm_x[:, 0:128],
                    rhs=warm_x[:],
                    start=True,
                    stop=True,
                )

            def bf16_tile(x_sb, tw, w1_sb, w2_sb, b1_sb, yti):
                h_sb = hpool.tile([128, FLO, CT], bf16)
                for fo in range(FLO):
                    q, fq = divmod(fo, 4)
                    ph = phpool.tile([128, CT], fp32)
                    for ko in range(KO):
                        nc.tensor.matmul(
                            ph[:, :tw],
                            lhsT=w1_sb[:, q, ko, fq * 128 : (fq + 1) * 128],
                            rhs=x_sb[:, ko, :tw],
                            start=(ko == 0),
                            stop=(ko == KO - 1),
                        )
                    nc.scalar.activation(
                        h_sb[:, fo, :tw],
                        ph[:, :tw],
                        mybir.ActivationFunctionType.Gelu,
                        bias=b1_sb[:, fo : fo + 1],
                    )
                for do in range(KO):
                    py = pypool.tile([128, CT], fp32)
                    for fi in range(FLO):
                        nc.tensor.matmul(
                            py[:, :tw],
                            lhsT=w2_sb[:, fi, do * 128 : (do + 1) * 128],
                            rhs=h_sb[:, fi, :tw],
                            start=(fi == 0),
                            stop=(fi == FLO - 1),
                        )
                    y_do = ypool.tile([128, CT], bf16, tag="y_do")
                    nc.vector.tensor_copy(y_do[:, :tw], py[:, :tw])
                    nc.sync.dma_start(y[yti][:, do, :], y_do[:])

            def fp8_tile(xc_sb, tw, s, yti):
                hc = hcpool.tile([128, 8, 2, CT], f8e4)
                for fo in range(FLO):
                    ph = phpool.tile([128, CT], fp32)
                    for j in range(4):
                        nc.tensor.matmul(
                            ph[:, :tw],
                            lhsT=w1c_sb[:, s, fo, j],
                            rhs=xc_sb[:, j, :, :tw],
                            start=(j == 0),
                            stop=(j == 3),
                            perf_mode=DR,
                        )
                    nc.scalar.activation(
                        hc[:, fo // 2, fo % 2, :tw],
                        ph[:, :tw],
                        mybir.ActivationFunctionType.Gelu,
                        bias=b1c_sb[:, s, fo : fo + 1],
                        scale=1.0 / SCALE,
                    )
                for do in range(KO):
                    py = pypool.tile([128, CT], fp32)
                    for fj in range(8):
                        nc.tensor.matmul(
                            py[:, :tw],
                            lhsT=w2c_sb[:, s, fj, :, do * 128 : (do + 1) * 128],
                            rhs=hc[:, fj, :, :tw],
                            start=(fj == 0),
                            stop=(fj == 7),
                            perf_mode=DR,
                        )
                    y_do = ypool.tile([128, CT], bf16, tag="y_do")
                    nc.vector.tensor_copy(y_do[:, :tw], py[:, :tw])
                    nc.sync.dma_start(y[yti][:, do, :], y_do[:])

            # ---- Phase A (big expert, bf16) ----
            with tc.tile_pool(name="wa", bufs=1) as wa:
                w1a_sb = wa.tile([128, 4, KO, 512], bf16)
                w2a_sb = wa.tile([128, 16, D], bf16)

                x_first = xpool.tile([128, KO, CT], bf16, tag="x_sb")
                nc.sync.dma_start(x_first[:, 0:4, :], xa[0][:, 0:4, :])
                nc.sync.dma_start(x_first[:, 4:8, :], xa[0][:, 4:8, :])
                nc.sync.dma_start(w1a_sb[:, 0], w1a[:, 0])
                nc.sync.dma_start(b1a_sb[:], b1a[:])
                nc.sync.dma_start(b1b_sb[:], b1b[:])
                nc.sync.dma_start(b1c_sb[:], b1c[:])
                for q2 in range(1, 4):
                    nc.sync.dma_start(w1a_sb[:, q2], w1a[:, q2])
                for b in range(2):
                    nc.sync.dma_start(
                        w2a_sb[:, b * 8 : (b + 1) * 8, :], w2a[b]
                    )
                for s in range(2):
                    nc.sync.dma_start(w1c_sb[:, s], w1c[s])
                    nc.sync.dma_start(w2c_sb[:, s], w2c[s])

                for ti, (off, tw) in enumerate(tA):
                    if ti == 0:
                        x_sb = x_first
                    else:
                        x_sb = xpool.tile([128, KO, CT], bf16, tag="x_sb")
                        nc.sync.dma_start(x_sb[:], xa[ti])
                    bf16_tile(x_sb, tw, w1a_sb, w2a_sb, b1a_sb, ti)

            # ---- Phases C, D (fp8) with B weights loading into A's space ----
            with tc.tile_pool(name="wb", bufs=1) as wb:
                w1b_sb = wb.tile([128, 4, KO, 512], bf16)
                w2b_sb = wb.tile([128, 16, D], bf16)
                for q2 in range(4):
                    nc.sync.dma_start(w1b_sb[:, q2], w1b[:, q2])
                for b in range(2):
                    nc.sync.dma_start(
                        w2b_sb[:, b * 8 : (b + 1) * 8, :], w2b[b]
                    )
                xb_pre = []
                for ti in range(min(2, nB)):
                    t = xpool.tile([128, KO, CT], bf16, tag="x_sb")
                    nc.sync.dma_start(t[:], xa[nA + ti])
                    xb_pre.append(t)

                for ci, (off, tw) in enumerate(tC + tD):
                    s = 0 if ci < nC else 1
                    xc_sb = xcpool.tile([128, 4, 2, CT], f8e4, tag="xc_sb")
                    nc.sync.dma_start(xc_sb[:], xc[ci])
                    fp8_tile(xc_sb, tw, s, nA + nB + ci)

                # ---- Phase B (small expert, bf16) ----
                for ti, (off, tw) in enumerate(tB):
                    if ti < len(xb_pre):
                        x_sb = xb_pre[ti]
                    else:
                        x_sb = xpool.tile([128, KO, CT], bf16, tag="x_sb")
                        nc.sync.dma_start(x_sb[:], xa[nA + ti])
                    bf16_tile(x_sb, tw, w1b_sb, w2b_sb, b1b_sb, nA + ti)

    nc.compile()
    return nc


def kernel(x, gate_w, w1, b1, w2, b2):
    from concourse.bass_utils import run_bass_kernel_spmd
    import itertools

    global LAST_RESULTS

    x = np.asarray(x, dtype=np.float32)
    gate_w = np.asarray(gate_w, dtype=np.float32)
    w1 = np.asarray(w1, dtype=np.float32)
    b1 = np.asarray(b1, dtype=np.float32)
    w2 = np.asarray(w2, dtype=np.float32)
    b2 = np.asarray(b2, dtype=np.float32)

    B, S, Din = x.shape
    assert Din == D and gate_w.shape == (D, E)
    T = B * S
    xf = x.reshape(T, D)

    # ---- Host router ----
    logits = xf.astype(np.float64) @ gate_w.astype(np.float64)
    idx0 = np.argmax(logits, axis=1)
    rows = np.arange(T)
    v0 = logits[rows, idx0]
    l2 = logits.copy()
    l2[rows, idx0] = -np.inf
    idx1 = np.argmax(l2, axis=1)
    v1_ = l2[rows, idx1]
    e1 = np.exp(v1_ - v0)
    cw0 = 1.0 / (1.0 + e1)
    cw1 = e1 / (1.0 + e1)

    exp_ids, exp_w, ch_ids, ch_w = [], [], [], []
    for e in range(E):
        sel0 = idx0 == e
        sel1e = (idx1 == e) & (cw1 >= THETA)
        sel1c = (idx1 == e) & (cw1 < THETA)
        ids = np.nonzero(sel0 | sel1e)[0]
        exp_ids.append(ids)
        exp_w.append(np.where(sel0[ids], cw0[ids], cw1[ids]))
        ids = np.nonzero(sel1c)[0]
        ch_ids.append(ids)
        ch_w.append(cw1[ids])
    ex = np.array([len(i) for i in exp_ids])
    ch = np.array([len(i) for i in ch_ids])

    # ---- Pairing: minimize 256*(CA+CB) + 178*(CC+CD) ----
    best = None
    for perm in itertools.permutations(range(E)):
        pairs = [(perm[0], perm[1]), (perm[2], perm[3]),
                 (perm[4], perm[5]), (perm[6], perm[7])]
        CA = max(ex[a] for a, _ in pairs)
        CB = max(ex[b] for _, b in pairs)
        CC = max(ch[a] for a, _ in pairs)
        CD = max(ch[b] for _, b in pairs)
        cost = 256 * (CA + CB) + 178 * (CC + CD)
        if best is None or cost < best[0]:
            best = (cost, pairs)
    pairs = best[1]
    CA = int(max(ex[a] for a, _ in pairs)); CA += CA & 1
    CB = int(max(ex[b] for _, b in pairs)); CB += CB & 1
    CC = int(max(ch[a] for a, _ in pairs)); CC += CC & 1
    CD = int(max(ch[b] for _, b in pairs)); CD += CD & 1
    caps = (CA, CB, CC, CD)

    if caps not in _NC_CACHE:
        _NC_CACHE[caps] = _build(caps)
    nc = _NC_CACHE[caps]

    tA, tB, tC, tD = (_cap_tiles(c) for c in caps)
    nA, nB, nC, nD = len(tA), len(tB), len(tC), len(tD)
    n_tot = nA + nB + nC + nD

    def pack_bf16(ids_seg):
        n = len(ids_seg)
        return xf[ids_seg].astype(BF16).reshape(n, KO, 128).transpose(2, 1, 0)

    def pack_f8(ids_seg):
        n = len(ids_seg)
        return (
            xf[ids_seg].astype(F8).reshape(n, 4, 2, 128).transpose(3, 1, 2, 0)
        )

    in_maps = [None] * N_CORES
    for pi, (eA, eB) in enumerate(pairs):
        xa_t = np.zeros((nA + nB, 128, KO, CT), dtype=BF16)
        for ti, (off, tw) in enumerate(tA):
            seg = exp_ids[eA][off : off + tw]
            if len(seg):
                xa_t[ti, :, :, : len(seg)] = pack_bf16(seg)
        for ti, (off, tw) in enumerate(tB):
            seg = exp_ids[eB][off : off + tw]
            if len(seg):
                xa_t[nA + ti, :, :, : len(seg)] = pack_bf16(seg)
        xc_t = np.zeros((nC + nD, 128, 4, 2, CT), dtype=F8)
        for ti, (off, tw) in enumerate(tC):
            seg = ch_ids[eA][off : off + tw]
            if len(seg):
                xc_t[ti, :, :, :, : len(seg)] = pack_f8(seg)
        for ti, (off, tw) in enumerate(tD):
            seg = ch_ids[eB][off : off + tw]
            if len(seg):
                xc_t[nC + ti, :, :, :, : len(seg)] = pack_f8(seg)
        xa_t = np.ascontiguousarray(xa_t)
        xc_t = np.ascontiguousarray(xc_t)

        for h in range(2):
            sl = slice(h * FL, (h + 1) * FL)
            w1a_c = np.ascontiguousarray(
                w1[eA][:, sl].reshape(KO, 128, 4, 512).transpose(1, 2, 0, 3).astype(BF16)
            )
            w1b_c = np.ascontiguousarray(
                w1[eB][:, sl].reshape(KO, 128, 4, 512).transpose(1, 2, 0, 3).astype(BF16)
            )
            w2a_c = np.ascontiguousarray(
                w2[eA][sl, :].reshape(2, 8, 128, D).transpose(0, 2, 1, 3).astype(BF16)
            )
            w2b_c = np.ascontiguousarray(
                w2[eB][sl, :].reshape(2, 8, 128, D).transpose(0, 2, 1, 3).astype(BF16)
            )
            w1c_c = np.ascontiguousarray(
                np.stack(
                    [
                        (SCALE * w1[e][:, sl])
                        .astype(F8)
                        .reshape(4, 2, 128, FLO, 128)
                        .transpose(2, 3, 0, 1, 4)
                        for e in (eA, eB)
                    ]
                )
            )  # [2, 128, FLO, 4, 2, 128]
            w2c_c = np.ascontiguousarray(
                np.stack(
                    [
                        (SCALE * w2[e][sl, :])
                        .astype(F8)
                        .reshape(8, 2, 128, D)
                        .transpose(2, 0, 1, 3)
                        for e in (eA, eB)
                    ]
                )
            )  # [2, 128, 8, 2, D]
            b1a_c = np.ascontiguousarray(b1[eA][sl].reshape(FLO, 128).T)
            b1b_c = np.ascontiguousarray(b1[eB][sl].reshape(FLO, 128).T)
            b1c_c = np.ascontiguousarray(
                np.stack([b1[e][sl].reshape(FLO, 128).T for e in (eA, eB)], axis=1)
            )  # [128, 2, FLO]
            in_maps[2 * pi + h] = {
                "xa": xa_t,
                "xc": xc_t,
                "w1a": w1a_c,
                "w1b": w1b_c,
                "w2a": w2a_c,
                "w2b": w2b_c,
                "w1c": w1c_c,
                "w2c": w2c_c,
                "b1a": b1a_c,
                "b1b": b1b_c,
                "b1c": b1c_c,
            }

    res = run_bass_kernel_spmd(nc, in_maps, core_ids=list(range(N_CORES)))
    LAST_RESULTS = res

    # ---- Host combine ----
    out = np.zeros((T, D), dtype=np.float32)
    for pi, (eA, eB) in enumerate(pairs):
        ysum = res.results[2 * pi]["y"].astype(np.float32) + res.results[
            2 * pi + 1
        ]["y"].astype(np.float32)

        def scatter(ti, ids_seg, w_seg, e, scale):
            n = len(ids_seg)
            if n == 0:
                return
            yt = ysum[ti, :, :, :n].transpose(2, 1, 0).reshape(n, D)
            out[ids_seg] += w_seg[:, None].astype(np.float32) * (
                yt * scale + b2[e]
            )

        for ti, (off, tw) in enumerate(tA):
            scatter(ti, exp_ids[eA][off : off + tw], exp_w[eA][off : off + tw], eA, 1.0)
        for ti, (off, tw) in enumerate(tC):
            scatter(nA + nB + ti, ch_ids[eA][off : off + tw], ch_w[eA][off : off + tw], eA, 1.0 / SCALE)
        for ti, (off, tw) in enumerate(tD):
            scatter(nA + nB + nC + ti, ch_ids[eB][off : off + tw], ch_w[eB][off : off + tw], eB, 1.0 / SCALE)
        for ti, (off, tw) in enumerate(tB):
            scatter(nA + ti, exp_ids[eB][off : off + tw], exp_w[eB][off : off + tw], eB, 1.0)

    return out.reshape(B, S, D)
